# revision 1
# baseline (speedup 1.0000x reference)
"""Trainium2 kernel for nn_AggrEncoder (segment-max + BN + 1x1 conv + fc).

Sharding: pure data-parallel over batch, 4 rows/core on 8 cores.

Host prep (sharding/layout): computes each row's counting-sort order of the
time axis by window id and ships the feature rows pre-permuted into that
order (tiled [128, 32, 128] so sorted position = g*128 + p), plus the
segment-boundary flag vector. BN+conv+fc fold into one (128->8) affine
W_eff/b_eff. (The device-side gather primitives - dma_gather /
indirect_copy / ap_gather custom GPSIMD ucode - fail in this environment,
so the permutation is applied host-side; the device still streams every
payload byte and performs the entire reduction + matmul chain.)

Device per row (chunk-pipelined in 8 chunks of 512 sorted columns so DMA,
DVE scan, PE matmul and evacuation overlap):
  1. DMA sorted features, pre-transposed [128 d, 4096 sorted-t], per chunk.
  2. PE ones-matmul replicates boundary flags to all partitions (bf16),
     per chunk, evacuated by ACT.
  3. DVE tensor_tensor_scan(op0=mult, op1=max) per chunk, chained via
     `initial`: segmented running max along the sorted axis; each window's
     max (clamped at 0, matching the reference's zeros-init scatter-max)
     sits at its segment-end column.
  4. PE matmul W_eff^T @ S_chunk -> PSUM [8, 512]; bias folded into the
     PSUM evacuation (ACT/DVE split); DMA out [8, 4096].
Host unshard: picks the 512 segment-end columns per row (b_eff for empty
windows), transposes to (Tu, 8), concatenates rows.
"""

import sys

import numpy as np

for _p in ("/opt/trn_rl_repo",):
    if _p not in sys.path:
        sys.path.insert(0, _p)

import concourse.bass as bass
import concourse.bacc as bacc
import concourse.mybir as mybir
from concourse import bass_utils
from concourse._compat import get_trn_type
from concourse.tile import TileContext

import ml_dtypes

B, T, D, Tu, Dout, M = 32, 4096, 128, 512, 64, 8
NCORES = 8
RPC = B // NCORES  # rows per core
BN_EPS = 1e-5

_CACHE = {}


def build_bass():
    import os
    f32r_tail = os.environ.get("KV_F32R_TAIL", "0") == "1"
    f32r_tp = os.environ.get("KV_F32R_TP", "0") == "1"
    bcast = os.environ.get("KV_BCAST", "0") == "1"
    bufs_sb = int(os.environ.get("KB_SB", "2"))
    bufs_pt = int(os.environ.get("KB_PT", "3"))
    ka_noscan = os.environ.get("KA_NOSCAN", "0") == "1"
    ka_notail = os.environ.get("KA_NOTAIL", "0") == "1"
    ka_nob01 = os.environ.get("KA_NOB01", "0") == "1"
    ka_nodma = os.environ.get("KA_NODMA", "0") == "1"
    evd = int(os.environ.get("KB_EVD", "2"))  # of 8 tail evacs, run on DVE
    nc = bacc.Bacc(get_trn_type() or "TRN2", target_bir_lowering=False)

    fsort = nc.dram_tensor(
        "fsort", [RPC, D, T], mybir.dt.float32, kind="ExternalInput"
    )
    b01 = nc.dram_tensor("b01", [RPC, 1, T], mybir.dt.bfloat16, kind="ExternalInput")
    weff_dt = mybir.dt.float32r if f32r_tail else mybir.dt.float32
    wefft = nc.dram_tensor("wefft", [D, M], weff_dt, kind="ExternalInput")
    beff = nc.dram_tensor("beff", [M, 1], mybir.dt.float32, kind="ExternalInput")
    ones1 = nc.dram_tensor("ones1", [1, 128], mybir.dt.bfloat16, kind="ExternalInput")
    out = nc.dram_tensor("out", [RPC, M, T], mybir.dt.float32, kind="ExternalOutput")

    NQ = T // 512  # 8 chunks of 512 sorted columns

    with TileContext(nc) as tc:
        with (
            tc.tile_pool(name="const", bufs=1) as cpool,
            tc.tile_pool(name="idxp", bufs=2) as idxp,
            tc.tile_pool(name="gpool", bufs=bufs_sb) as gpool,
            tc.tile_pool(name="vpool", bufs=bufs_sb) as vpool,
            tc.tile_pool(name="spool", bufs=bufs_sb) as spool,
            tc.tile_pool(name="opool", bufs=bufs_sb) as opool,
            tc.tile_pool(name="psum_b", bufs=2, space="PSUM") as ppb,
            tc.tile_pool(name="psum_o", bufs=2, space="PSUM") as ppo,
        ):
            ones_sb = cpool.tile([1, 128], mybir.dt.bfloat16, tag="ones")
            nc.sync.dma_start(ones_sb[:], ones1[:])
            weff_sb = cpool.tile([D, M], weff_dt, tag="weff")
            nc.sync.dma_start(weff_sb[:], wefft[:])
            beff_sb = cpool.tile([M, 1], mybir.dt.float32, tag="beff")
            nc.sync.dma_start(beff_sb[:], beff[:])
            zeros8 = cpool.tile([M, 512], mybir.dt.float32, tag="z8")
            nc.vector.memset(zeros8[:], 0.0)

            for r in range(RPC):
                b01_sb = idxp.tile([1, T], mybir.dt.bfloat16, tag="b01")
                nc.sync.dma_start(b01_sb[:], b01[r])

                # 1. sorted features, pre-transposed [d, sorted-t];
                # DMA'd in 512-column chunks so the scan can start early
                FT = gpool.tile([D, T], mybir.dt.float32, tag="FT")
                if not ka_nodma:
                    for q in range(NQ):
                        nc.sync.dma_start(
                            FT[:, q * 512:(q + 1) * 512],
                            fsort[r][:, q * 512:(q + 1) * 512],
                        )
                else:
                    nc.vector.memset(FT[:, 0:8], 0.0)

                # 3. replicate boundary flags across partitions (PE), or read
                # them with a 0-stride partition-broadcast AP (KV_BCAST=1)
                b01rep = vpool.tile([128, T], mybir.dt.bfloat16, tag="b01rep")
                if ka_nob01:
                    nc.vector.memset(b01rep[:, 0:8], 0.0)
                else:
                    for h in range(NQ):
                        pb = ppb.tile([128, 512], mybir.dt.float32, tag="pb")
                        nc.tensor.matmul(
                            pb[:],
                            ones_sb[:],
                            b01_sb[:, h * 512:(h + 1) * 512],
                            start=True,
                            stop=True,
                        )
                        nc.scalar.copy(b01rep[:, h * 512:(h + 1) * 512], pb[:])

                # 4+5. chunk-pipelined: scan chunk q (chained via initial),
                # then immediately the tail matmul + bias evac for chunk q
                S = spool.tile([128, T], mybir.dt.float32, tag="S")
                out_sb = opool.tile([M, T], mybir.dt.float32, tag="osb")
                if ka_noscan:
                    nc.vector.memset(S[:, 0:8], 0.0)
                if ka_notail:
                    nc.vector.memset(out_sb[:], 0.0)
                for h in range(NQ):
                    lo, hi = h * 512, (h + 1) * 512
                    if not ka_noscan:
                        nc.vector.tensor_tensor_scan(
                            S[:, lo:hi],
                            b01rep[:, lo:hi],
                            FT[:, lo:hi],
                            0.0 if h == 0 else S[:, lo - 1:lo],
                            op0=mybir.AluOpType.mult,
                            op1=mybir.AluOpType.max,
                        )
                    if ka_notail:
                        continue
                    po = ppo.tile([M, 512], mybir.dt.float32, tag="po")
                    mm_rhs = S[:, lo:hi]
                    if f32r_tail:
                        mm_rhs = mm_rhs.bitcast(mybir.dt.float32r)
                    nc.tensor.matmul(po[:], weff_sb[:], mm_rhs, start=True, stop=True)
                    if h >= evd:
                        nc.scalar.add(out_sb[:, lo:hi], po[:], beff_sb[:])
                    else:
                        nc.vector.scalar_tensor_tensor(
                            out_sb[:, lo:hi],
                            po[:],
                            beff_sb[:],
                            zeros8[:],
                            op0=mybir.AluOpType.add,
                            op1=mybir.AluOpType.add,
                        )
                nc.sync.dma_start(out[r], out_sb[:])

    if not nc.is_finalized():
        nc.finalize()
    return nc


def _host_prep(x, mask, tw_uniq, bn_gamma, bn_beta, bn_mean, bn_var,
               conv_w, conv_b, fc_w, fc_b):
    tw = x[:, :, 0]
    u0 = tw_uniq[:, 0, 0]
    idx = np.clip((tw - u0[:, None]).astype(np.int32), 0, Tu - 1)
    idx = np.where(mask[:, :, 0], idx, Tu)  # masked -> trash segment

    fsort = np.empty((B, D, T), np.float32)
    b01 = np.empty((B, 1, T), ml_dtypes.bfloat16)
    epos = np.empty((B, Tu), np.int64)
    for b in range(B):
        perm = np.argsort(idx[b], kind="stable")
        sidx = idx[b][perm]
        fsort[b] = x[b][:, 1:][perm].T  # [d, sorted-t]
        bb = np.empty(T, np.float32)
        bb[0] = 0.0
        bb[1:] = (sidx[1:] == sidx[:-1]).astype(np.float32)
        b01[b, 0] = bb.astype(ml_dtypes.bfloat16)
        counts = np.bincount(sidx, minlength=Tu + 1)[:Tu]
        starts = np.concatenate([[0], np.cumsum(counts)[:-1]])
        epos[b] = np.where(counts > 0, starts + counts - 1, -1)

    s = (bn_gamma.astype(np.float64)
         / np.sqrt(bn_var.astype(np.float64) + BN_EPS))
    t_aff = bn_beta.astype(np.float64) - bn_mean.astype(np.float64) * s
    wc = fc_w.astype(np.float64) @ conv_w.astype(np.float64)  # (8, 128)
    w_eff = wc * s[None, :]
    b_eff = (fc_w.astype(np.float64)
             @ (conv_w.astype(np.float64) @ t_aff + conv_b.astype(np.float64))
             + fc_b.astype(np.float64))
    wefft = np.ascontiguousarray(w_eff.T.astype(np.float32))  # (128, 8)
    beff = b_eff.astype(np.float32).reshape(M, 1)
    return fsort, b01, epos, wefft, beff


def _build_in_maps(fsort, b01, wefft, beff):
    ones1 = np.ones((1, 128), ml_dtypes.bfloat16)
    in_maps = []
    for c in range(NCORES):
        rows = slice(c * RPC, (c + 1) * RPC)
        in_maps.append(dict(
            fsort=fsort[rows],
            b01=b01[rows],
            wefft=wefft,
            beff=beff,
            ones1=ones1,
        ))
    return in_maps


def _unshard(core_outs, epos, beff):
    final = np.empty((B, Tu, M), np.float32)
    for c in range(NCORES):
        of = core_outs[c]  # (RPC, M, T)
        for r in range(RPC):
            b = c * RPC + r
            ep = epos[b]
            cols = of[r][:, np.where(ep >= 0, ep, 0)].T  # (Tu, M)
            final[b] = np.where((ep >= 0)[:, None], cols, beff[:, 0][None, :])
    return final


def kernel(x, mask, tw_uniq, bn_gamma, bn_beta, bn_mean, bn_var,
           conv_w, conv_b, fc_w, fc_b):
    fsort, b01, epos, wefft, beff = _host_prep(
        x, mask, tw_uniq, bn_gamma, bn_beta, bn_mean, bn_var,
        conv_w, conv_b, fc_w, fc_b)

    if "nc" not in _CACHE:
        _CACHE["nc"] = build_bass()
    nc = _CACHE["nc"]

    in_maps = _build_in_maps(fsort, b01, wefft, beff)
    res = bass_utils.run_bass_kernel_spmd(nc, in_maps, list(range(NCORES)))
    core_outs = [res.results[c]["out"] for c in range(NCORES)]
    return _unshard(core_outs, epos, beff)



# revision 4
# speedup vs baseline: 1.7166x; 1.7166x over previous
"""Trainium2 kernel for nn_AggrEncoder (segment-max + BN + 1x1 conv + fc).

Sharding: pure data-parallel over batch, 4 rows/core on 8 cores.

Host prep (layout only): per batch row, counting-sorts the valid (masked-in)
time columns by window id, pads each window's group to even length and splits
it across two half-arrays A|B at identical pair-positions, so a single
tensor_tensor max (the "fold", 2x DVE throughput in bf16) reduces the row to
T2 pair-columns with every window still contiguous. BN+conv+fc fold into one
(128->8) affine W_eff/b_eff. All feature payload ships in bf16 (rel-err
budget 2e-2; measured ~4e-3).

Device per row (rows pipelined; DVE is the bottleneck engine ~14us):
  1. DMA sorted features [128, 2*T2] bf16 (A half | B half).
  2. Fold on DVE: G = max(A, B)  (bf16 tensor_tensor -> 2x mode).
  3. PE ones-matmul replicates the pair-level segment-continuation flags to
     128 partitions into PSUM f32, two 512-col (bank) chunks per scan chunk.
  4. DVE tensor_tensor_scan (op0=mult, op1=max) over G, flags read directly
     from PSUM, carry chained across 1024-col chunks via `initial`. Flag=0 at
     window starts resets the carry and applies the reference's max(0, .)
     clamp for free (zeros-init scatter-max semantics).
  5. Tail: 4 matmuls W_eff^T @ S_slice -> po[8, T2] PSUM, one bank each.
  6. ACT evacuation: per-partition bias add + f32->bf16 in one scalar.add;
     DMA out [8, T2] bf16.
Host unshard: picks each window's segment-end pair column; empty windows get
b_eff.
"""

import sys

import numpy as np

for _p in ("/opt/trn_rl_repo",):
    if _p not in sys.path:
        sys.path.insert(0, _p)

import concourse.bass as bass
import concourse.bacc as bacc
import concourse.mybir as mybir
from concourse import bass_utils
from concourse._compat import get_trn_type
from concourse.tile import TileContext

import ml_dtypes

B, T, D, Tu, Dout, M = 32, 4096, 128, 512, 64, 8
NCORES = 8
RPC = B // NCORES  # rows per core
BN_EPS = 1e-5

T2 = 2048          # pair columns per row (folded length)
SC = 1024          # scan chunk (DVE) — 2 PSUM banks of f32 flags
BK = 512           # PSUM bank width in f32 — matmul output limit

_CACHE = {}


def build_bass():
    nc = bacc.Bacc(get_trn_type() or "TRN2", target_bir_lowering=False)

    fsort = nc.dram_tensor(
        "fsort", [RPC, D, 2 * T2], mybir.dt.bfloat16, kind="ExternalInput"
    )
    b01 = nc.dram_tensor(
        "b01", [1, RPC * T2], mybir.dt.bfloat16, kind="ExternalInput"
    )
    # cols 0:8 = W_eff^T (bf16); cols 8:10 = per-partition f32 bias (bitcast)
    weffb = nc.dram_tensor("weffb", [D, 10], mybir.dt.bfloat16, kind="ExternalInput")
    out = nc.dram_tensor(
        "out", [RPC, M, T2], mybir.dt.bfloat16, kind="ExternalOutput"
    )

    with TileContext(nc) as tc:
        with (
            tc.tile_pool(name="const", bufs=1) as cpool,
            tc.tile_pool(name="gpool", bufs=2) as gpool,
            tc.tile_pool(name="fpool", bufs=2) as fpool,
            tc.tile_pool(name="spool", bufs=2) as spool,
            tc.tile_pool(name="opool", bufs=2) as opool,
            tc.tile_pool(name="psum_b", bufs=2, space="PSUM") as ppb,
            tc.tile_pool(name="psum_o", bufs=1, space="PSUM") as ppo,
        ):
            b01_sb = cpool.tile([1, RPC * T2], mybir.dt.bfloat16, tag="b01")
            nc.sync.dma_start(b01_sb[:], b01[:])
            weffb_sb = cpool.tile([D, 10], mybir.dt.bfloat16, tag="weffb")
            nc.sync.dma_start(weffb_sb[:], weffb[:])
            ones_sb = cpool.tile([1, 128], mybir.dt.bfloat16, tag="ones")
            nc.vector.memset(ones_sb[:], 1.0)

            weff_ap = weffb_sb[:, 0:8]
            bias_ap = weffb_sb[:, 8:10].bitcast(mybir.dt.float32)

            for r in range(RPC):
                FT = gpool.tile([D, 2 * T2], mybir.dt.bfloat16, tag="FT")
                nc.sync.dma_start(FT[:, 0:T2], fsort[r][:, 0:T2])
                nc.sync.dma_start(FT[:, T2:2 * T2], fsort[r][:, T2:2 * T2])

                # fold: G = max(A, B); bf16 tensor_tensor runs 2x on DVE
                G = fpool.tile([D, T2], mybir.dt.bfloat16, tag="G")
                nc.vector.tensor_tensor(
                    G[:], FT[:, 0:T2], FT[:, T2:2 * T2], op=mybir.AluOpType.max
                )

                # segmented scan, chunked by SC with PSUM flags
                S = spool.tile([D, T2], mybir.dt.bfloat16, tag="S")
                for q in range(T2 // SC):
                    lo = q * SC
                    pb = ppb.tile([128, SC], mybir.dt.float32, tag="pb")
                    for h in range(SC // BK):
                        fl_lo = r * T2 + lo + h * BK
                        nc.tensor.matmul(
                            pb[:, h * BK:(h + 1) * BK],
                            ones_sb[:],
                            b01_sb[:, fl_lo:fl_lo + BK],
                            start=True,
                            stop=True,
                        )
                    nc.vector.tensor_tensor_scan(
                        S[:, lo:lo + SC],
                        pb[:],
                        G[:, lo:lo + SC],
                        0.0 if q == 0 else S[:, lo - 1:lo],
                        op0=mybir.AluOpType.mult,
                        op1=mybir.AluOpType.max,
                    )

                # tail: po[8, T2] across 4 PSUM banks, one matmul per bank
                po = ppo.tile([M, T2], mybir.dt.float32, tag="po")
                for g in range(T2 // BK):
                    nc.tensor.matmul(
                        po[:, g * BK:(g + 1) * BK],
                        weff_ap,
                        S[:, g * BK:(g + 1) * BK],
                        start=True,
                        stop=True,
                    )
                out_sb = opool.tile([M, T2], mybir.dt.bfloat16, tag="osb")
                nc.scalar.add(out_sb[:], po[:], bias_ap[0:M])
                nc.sync.dma_start(out[r], out_sb[:])

    if not nc.is_finalized():
        nc.finalize()
    return nc


def _host_prep(x, mask, tw_uniq, bn_gamma, bn_beta, bn_mean, bn_var,
               conv_w, conv_b, fc_w, fc_b):
    tw = x[:, :, 0]
    u0 = tw_uniq[:, 0, 0]
    idx = np.clip((tw - u0[:, None]).astype(np.int32), 0, Tu - 1)  # (B, T)
    valid = mask[:, :, 0]
    key = np.where(valid, idx, Tu).astype(np.int64)                # (B, T)

    featsT = np.ascontiguousarray(
        x[:, :, 1:].transpose(0, 2, 1)).astype(ml_dtypes.bfloat16)  # (B, D, T)

    fsort = np.zeros((B, D, 2 * T2), ml_dtypes.bfloat16)
    b01 = np.ones((B, T2), np.float32)
    epos = np.empty((B, Tu), np.int64)

    rowidx = np.arange(T)
    for b in range(B):
        counts = np.bincount(key[b], minlength=Tu + 1)[:Tu]        # (Tu,)
        h = (counts + 1) // 2
        hrow = int(h.sum())
        if hrow > T2:
            # overflow fallback (not expected for the reference's mask):
            # reduce this row host-side and ship one column per window.
            hidden = np.zeros((D, Tu), np.float32)
            fv = featsT[b].astype(np.float32)
            v = valid[b]
            np.maximum.at(hidden.T, idx[b][v], fv[:, v].T)
            fsort[b] = 0
            fsort[b][:, :Tu] = np.maximum(hidden, 0.0).astype(ml_dtypes.bfloat16)
            b01[b] = 1.0
            b01[b][:Tu] = 0.0
            epos[b] = np.where(counts > 0, np.arange(Tu), -1)
            continue

        order = np.argsort(key[b], kind="stable")                  # (T,)
        skey = key[b][order]
        nvalid = int((skey < Tu).sum())
        order = order[:nvalid]
        skey = skey[:nvalid]

        cstart = np.concatenate([[0], np.cumsum(counts)])          # (Tu+1,)
        rank = rowidx[:nvalid] - cstart[skey]                      # pos in segment
        pstart = np.concatenate([[0], np.cumsum(h)[:-1]])          # (Tu,)
        hseg = h[skey]
        in_a = rank < hseg
        paircol = pstart[skey] + np.where(in_a, rank, rank - hseg)
        col = np.where(in_a, paircol, T2 + paircol)

        fsort[b][:, col] = featsT[b][:, order]
        b01[b][pstart] = 0.0
        b01[b][hrow:] = 0.0
        epos[b] = np.where(counts > 0, pstart + h - 1, -1)

    s = (bn_gamma.astype(np.float64)
         / np.sqrt(bn_var.astype(np.float64) + BN_EPS))
    t_aff = bn_beta.astype(np.float64) - bn_mean.astype(np.float64) * s
    wc = fc_w.astype(np.float64) @ conv_w.astype(np.float64)       # (8, 128)
    w_eff = wc * s[None, :]
    b_eff = (fc_w.astype(np.float64)
             @ (conv_w.astype(np.float64) @ t_aff + conv_b.astype(np.float64))
             + fc_b.astype(np.float64)).astype(np.float32)         # (8,)

    # weffb: [128, 10] bf16; cols 0:8 = W_eff^T, cols 8:10 = f32 bias bitcast
    weffb = np.zeros((D, 10), ml_dtypes.bfloat16)
    weffb[:, 0:8] = w_eff.T.astype(ml_dtypes.bfloat16)
    bias128 = np.zeros((D,), np.float32)
    bias128[:M] = b_eff
    weffb[:, 8:10] = bias128.reshape(D, 1).view(ml_dtypes.bfloat16)

    return fsort, b01.astype(ml_dtypes.bfloat16), epos, weffb, b_eff


def _build_in_maps(fsort, b01, weffb):
    in_maps = []
    for c in range(NCORES):
        rows = slice(c * RPC, (c + 1) * RPC)
        in_maps.append(dict(
            fsort=fsort[rows],
            b01=b01[rows].reshape(1, RPC * T2),
            weffb=weffb,
        ))
    return in_maps


def _unshard(core_outs, epos, b_eff):
    final = np.empty((B, Tu, M), np.float32)
    for c in range(NCORES):
        of = core_outs[c]  # (RPC, M, T2) bf16
        for r in range(RPC):
            b = c * RPC + r
            ep = epos[b]
            cols = of[r][:, np.where(ep >= 0, ep, 0)].T.astype(np.float32)
            final[b] = np.where((ep >= 0)[:, None], cols, b_eff[None, :])
    return final


def kernel(x, mask, tw_uniq, bn_gamma, bn_beta, bn_mean, bn_var,
           conv_w, conv_b, fc_w, fc_b):
    fsort, b01, epos, weffb, b_eff = _host_prep(
        x, mask, tw_uniq, bn_gamma, bn_beta, bn_mean, bn_var,
        conv_w, conv_b, fc_w, fc_b)

    if "nc" not in _CACHE:
        _CACHE["nc"] = build_bass()
    nc = _CACHE["nc"]

    in_maps = _build_in_maps(fsort, b01, weffb)
    res = bass_utils.run_bass_kernel_spmd(nc, in_maps, list(range(NCORES)))
    core_outs = [res.results[c]["out"] for c in range(NCORES)]
    return _unshard(core_outs, epos, b_eff)


# revision 12
# speedup vs baseline: 2.0441x; 1.1908x over previous
"""Trainium2 kernel for nn_AggrEncoder (segment-max + BN + 1x1 conv + fc).

Sharding: pure data-parallel over batch, 4 rows/core on 8 cores.

Host prep (layout only): per batch row, counting-sorts the valid (masked-in)
time columns by window id, pads each window's group to even length and splits
it across two half-arrays A|B at identical pair-positions, so a tensor_tensor
max (the "fold", 2x DVE throughput in bf16) halves the row to T2 pair-columns
with every window still contiguous. A and B are interleaved in DRAM at chunk
granularity (512 pair-cols for rows 0/3, 1024 for rows 1/2) so each fold
chunk depends on a single DMA transfer. BN+conv+fc fold into one (128->8)
affine W_eff/b_eff. All feature payload ships in bf16 (rel-err budget 2e-2;
measured ~6e-3).

Device pipeline (DVE is the bottleneck engine; the tile_wait_until stamps pin
the scheduler to the software-pipelined order below — its readiness model
otherwise hoists all folds ahead of the scans and starves DVE):
  1. fsort DMAs on the SP sequencer (row 0 in four transfers for a fast
     start), b01/weffb on the ACT sequencer.
  2. Fold on DVE per chunk: G = max(A_c, B_c).
  3. PE ones-matmul replicates pair-level segment-continuation flags into
     PSUM f32 per scan chunk; flag chunks for row r+1 run ahead of row r
     tails.
  4. DVE tensor_tensor_scan (op0=mult, op1=max) over G, flags read straight
     from PSUM, carry chained across chunks via `initial`. Flag=0 at window
     starts resets the carry and applies the reference max(0, .) clamp.
  5. Tail: W_eff^T @ S per 512-col PSUM bank; ACT scalar.add evacuates each
     bank (bias + f32->bf16) as soon as it is ready.
  6. out DMAs on SP; the last row's is split in half to shorten the drain.
Host unshard: picks each window's segment-end pair column; empty windows get
b_eff.
"""

import sys

import numpy as np

for _p in ("/opt/trn_rl_repo",):
    if _p not in sys.path:
        sys.path.insert(0, _p)

import concourse.bass as bass
import concourse.bacc as bacc
import concourse.mybir as mybir
from concourse import bass_utils
from concourse._compat import get_trn_type
from concourse.tile import TileContext

import ml_dtypes

B, T, D, Tu, Dout, M = 32, 4096, 128, 512, 64, 8
NCORES = 8
RPC = B // NCORES  # rows per core
BN_EPS = 1e-5

T2 = 2048          # pair columns per row (folded length)
BK = 512           # PSUM bank width in f32 — matmul output limit

# per-row interleave grain (pair columns per fold/scan chunk)
ROW_GRAIN = (512, 1024, 1024, 512)

_CACHE = {}


def build_bass():
    nc = bacc.Bacc(get_trn_type() or "TRN2", target_bir_lowering=False)

    fsort = nc.dram_tensor(
        "fsort", [RPC, D, 2 * T2], mybir.dt.bfloat16, kind="ExternalInput"
    )
    b01 = nc.dram_tensor(
        "b01", [1, RPC * T2], mybir.dt.bfloat16, kind="ExternalInput"
    )
    # cols 0:8 = W_eff^T (bf16); cols 8:10 = per-partition f32 bias (bitcast)
    weffb = nc.dram_tensor("weffb", [D, 10], mybir.dt.bfloat16, kind="ExternalInput")
    out = nc.dram_tensor(
        "out", [RPC, M, T2], mybir.dt.bfloat16, kind="ExternalOutput"
    )

    with TileContext(nc) as tc:
        with (
            tc.tile_pool(name="const", bufs=1) as cpool,
            tc.tile_pool(name="gpool", bufs=4) as gpool,
            tc.tile_pool(name="fpool", bufs=2) as fpool,
            tc.tile_pool(name="spool", bufs=2) as spool,
            tc.tile_pool(name="opool", bufs=2) as opool,
            tc.tile_pool(name="pb512", bufs=2, space="PSUM") as ppb5,
            tc.tile_pool(name="pb1024", bufs=2, space="PSUM") as ppb10,
            tc.tile_pool(name="psum_o", bufs=2, space="PSUM") as ppo,
        ):
            b01_sb = cpool.tile([1, RPC * T2], mybir.dt.bfloat16, tag="b01")
            nc.scalar.dma_start(b01_sb[:], b01[:])
            weffb_sb = cpool.tile([D, 10], mybir.dt.bfloat16, tag="weffb")
            nc.scalar.dma_start(weffb_sb[:], weffb[:])
            ones_sb = cpool.tile([1, 128], mybir.dt.bfloat16, tag="ones")
            nc.vector.memset(ones_sb[:], 1.0)

            weff_ap = weffb_sb[:, 0:8]
            bias_ap = weffb_sb[:, 8:10].bitcast(mybir.dt.float32)[0:M]

            # ---- fsort DMAs (SP): row 0 in 4 transfers, rest in 2 ----
            FTs = []
            for r in range(RPC):
                FT = gpool.tile([D, 2 * T2], mybir.dt.bfloat16, tag="FT",
                                name=f"FT{r}")
                FTs.append(FT)
            for c in range(4):
                lo = c * 1024
                nc.sync.dma_start(FTs[0][:, lo:lo + 1024],
                                  fsort[0][:, lo:lo + 1024])
            for r in range(1, RPC):
                nc.sync.dma_start(FTs[r][:, 0:T2], fsort[r][:, 0:T2])
                nc.sync.dma_start(FTs[r][:, T2:2 * T2], fsort[r][:, T2:2 * T2])

            # ---- per-row structures ----
            Gs, Ss, outs = [], [], []
            for r in range(RPC):
                G = fpool.tile([D, T2], mybir.dt.bfloat16, tag="G", name=f"G{r}")
                S = spool.tile([D, T2], mybir.dt.bfloat16, tag="S", name=f"S{r}")
                o = opool.tile([M, T2], mybir.dt.bfloat16, tag="osb",
                               name=f"osb{r}")
                Gs.append(G)
                Ss.append(S)
                outs.append(o)
            pbs = {}
            pos = {}

            def fold(r, c):
                gr = ROW_GRAIN[r]
                lo = c * gr
                nc.vector.tensor_tensor(
                    Gs[r][:, lo:lo + gr],
                    FTs[r][:, 2 * lo:2 * lo + gr],
                    FTs[r][:, 2 * lo + gr:2 * lo + 2 * gr],
                    op=mybir.AluOpType.max)

            def reps(r, c):
                gr = ROW_GRAIN[r]
                pool = ppb5 if gr == 512 else ppb10
                pb = pool.tile([128, gr], mybir.dt.float32, tag=f"pb{gr}",
                               name=f"pb{r}_{c}")
                pbs[(r, c)] = pb[:]
                for h in range(gr // BK):
                    fl_lo = r * T2 + c * gr + h * BK
                    nc.tensor.matmul(
                        pb[:, h * BK:(h + 1) * BK],
                        ones_sb[:],
                        b01_sb[:, fl_lo:fl_lo + BK],
                        start=True, stop=True)

            def scan(r, c):
                gr = ROW_GRAIN[r]
                lo = c * gr
                nc.vector.tensor_tensor_scan(
                    Ss[r][:, lo:lo + gr],
                    pbs[(r, c)],
                    Gs[r][:, lo:lo + gr],
                    0.0 if c == 0 else Ss[r][:, lo - 1:lo],
                    op0=mybir.AluOpType.mult,
                    op1=mybir.AluOpType.max)

            def tail(r, k):
                po = ppo.tile([M, BK], mybir.dt.float32, tag="po",
                              name=f"po{r}_{k}")
                pos[(r, k)] = po
                lo = k * BK
                nc.tensor.matmul(
                    po[:], weff_ap, Ss[r][:, lo:lo + BK],
                    start=True, stop=True)

            def evac(r, k):
                lo = k * BK
                nc.scalar.add(outs[r][:, lo:lo + BK], pos[(r, k)][:], bias_ap)

            def out_dma(r, h=None):
                if h is None:
                    nc.sync.dma_start(out[r], outs[r][:])
                else:
                    lo = h * 1024
                    nc.sync.dma_start(out[r][:, lo:lo + 1024],
                                      outs[r][:, lo:lo + 1024])

            # software-pipelined emission, pinned via tile_wait_until.
            # DVE order keeps every op's producer >= 2 slots back so the
            # carry/fold dependency latency (~250ns) hides behind the
            # intervening op.
            steps = [
                lambda: fold(0, 0), lambda: reps(0, 0),
                lambda: fold(0, 1), lambda: reps(0, 1),
                lambda: scan(0, 0),
                lambda: fold(0, 2), lambda: reps(0, 2),
                lambda: scan(0, 1), lambda: tail(0, 0), lambda: evac(0, 0),
                lambda: fold(0, 3), lambda: reps(0, 3),
                lambda: scan(0, 2), lambda: tail(0, 1), lambda: evac(0, 1),
                lambda: fold(1, 0), lambda: reps(1, 0),
                lambda: scan(0, 3), lambda: tail(0, 2), lambda: evac(0, 2),
                lambda: fold(1, 1), lambda: reps(1, 1),
                lambda: scan(1, 0),
                lambda: tail(0, 3), lambda: evac(0, 3), lambda: out_dma(0),
                lambda: fold(2, 0), lambda: reps(2, 0),
                lambda: scan(1, 1), lambda: tail(1, 0), lambda: evac(1, 0),
                lambda: fold(2, 1), lambda: reps(2, 1),
                lambda: scan(2, 0), lambda: tail(1, 1), lambda: evac(1, 1),
                lambda: tail(1, 2), lambda: evac(1, 2),
                lambda: fold(3, 0), lambda: reps(3, 0),
                lambda: scan(2, 1),
                lambda: tail(1, 3), lambda: evac(1, 3), lambda: out_dma(1),
                lambda: tail(2, 0), lambda: evac(2, 0),
                lambda: tail(2, 1), lambda: evac(2, 1),
                lambda: fold(3, 1), lambda: reps(3, 1),
                lambda: scan(3, 0), lambda: tail(2, 2), lambda: evac(2, 2),
                lambda: fold(3, 2), lambda: reps(3, 2),
                lambda: scan(3, 1),
                lambda: tail(2, 3), lambda: evac(2, 3), lambda: out_dma(2),
                lambda: fold(3, 3), lambda: reps(3, 3),
                lambda: scan(3, 2), lambda: tail(3, 0), lambda: evac(3, 0),
                lambda: tail(3, 1), lambda: evac(3, 1), lambda: out_dma(3, 0),
                lambda: scan(3, 3), lambda: tail(3, 2), lambda: evac(3, 2),
                lambda: tail(3, 3), lambda: evac(3, 3), lambda: out_dma(3, 1),
            ]
            import os
            force = os.environ.get("KW_FORCE", "1") == "1"
            for i, fn in enumerate(steps):
                with tc.tile_wait_until(i * 0.02, enable=force):
                    fn()

    if not nc.is_finalized():
        nc.finalize()
    return nc


def _host_prep(x, mask, tw_uniq, bn_gamma, bn_beta, bn_mean, bn_var,
               conv_w, conv_b, fc_w, fc_b):
    tw = x[:, :, 0]
    u0 = tw_uniq[:, 0, 0]
    idx = np.clip((tw - u0[:, None]).astype(np.int32), 0, Tu - 1)  # (B, T)
    valid = mask[:, :, 0]
    key = np.where(valid, idx, Tu).astype(np.int64)                # (B, T)

    featsT = np.ascontiguousarray(
        x[:, :, 1:].transpose(0, 2, 1)).astype(ml_dtypes.bfloat16)  # (B, D, T)

    fsort = np.zeros((B, D, 2 * T2), ml_dtypes.bfloat16)
    b01 = np.ones((B, T2), np.float32)
    epos = np.empty((B, Tu), np.int64)

    rowidx = np.arange(T)
    for b in range(B):
        gr = ROW_GRAIN[b % RPC]
        counts = np.bincount(key[b], minlength=Tu + 1)[:Tu]        # (Tu,)
        h = (counts + 1) // 2
        hrow = int(h.sum())
        if hrow > T2:
            # overflow fallback (not expected for the reference's mask):
            # reduce this row host-side and ship one column per window.
            hidden = np.zeros((D, Tu), np.float32)
            fv = featsT[b].astype(np.float32)
            v = valid[b]
            np.maximum.at(hidden.T, idx[b][v], fv[:, v].T)
            hb = np.maximum(hidden, 0.0).astype(ml_dtypes.bfloat16)
            fsort[b] = 0
            # A-columns of each chunk hold the values; B-columns stay 0
            pc = np.arange(Tu)
            acol = 2 * gr * (pc // gr) + (pc % gr)
            fsort[b][:, acol] = hb
            b01[b] = 1.0
            b01[b][:Tu] = 0.0
            epos[b] = np.where(counts > 0, np.arange(Tu), -1)
            continue

        order = np.argsort(key[b], kind="stable")                  # (T,)
        skey = key[b][order]
        nvalid = int((skey < Tu).sum())
        order = order[:nvalid]
        skey = skey[:nvalid]

        cstart = np.concatenate([[0], np.cumsum(counts)])          # (Tu+1,)
        rank = rowidx[:nvalid] - cstart[skey]                      # pos in segment
        pstart = np.concatenate([[0], np.cumsum(h)[:-1]])          # (Tu,)
        hseg = h[skey]
        in_a = rank < hseg
        paircol = pstart[skey] + np.where(in_a, rank, rank - hseg)
        base = 2 * gr * (paircol // gr) + (paircol % gr)
        col = np.where(in_a, base, base + gr)

        fsort[b][:, col] = featsT[b][:, order]
        b01[b][pstart] = 0.0
        b01[b][hrow:] = 0.0
        epos[b] = np.where(counts > 0, pstart + h - 1, -1)

    s = (bn_gamma.astype(np.float64)
         / np.sqrt(bn_var.astype(np.float64) + BN_EPS))
    t_aff = bn_beta.astype(np.float64) - bn_mean.astype(np.float64) * s
    wc = fc_w.astype(np.float64) @ conv_w.astype(np.float64)       # (8, 128)
    w_eff = wc * s[None, :]
    b_eff = (fc_w.astype(np.float64)
             @ (conv_w.astype(np.float64) @ t_aff + conv_b.astype(np.float64))
             + fc_b.astype(np.float64)).astype(np.float32)         # (8,)

    # weffb: [128, 10] bf16; cols 0:8 = W_eff^T, cols 8:10 = f32 bias bitcast
    weffb = np.zeros((D, 10), ml_dtypes.bfloat16)
    weffb[:, 0:8] = w_eff.T.astype(ml_dtypes.bfloat16)
    bias128 = np.zeros((D,), np.float32)
    bias128[:M] = b_eff
    weffb[:, 8:10] = bias128.reshape(D, 1).view(ml_dtypes.bfloat16)

    return fsort, b01.astype(ml_dtypes.bfloat16), epos, weffb, b_eff


def _build_in_maps(fsort, b01, weffb):
    in_maps = []
    for c in range(NCORES):
        rows = slice(c * RPC, (c + 1) * RPC)
        in_maps.append(dict(
            fsort=fsort[rows],
            b01=b01[rows].reshape(1, RPC * T2),
            weffb=weffb,
        ))
    return in_maps


def _unshard(core_outs, epos, b_eff):
    final = np.empty((B, Tu, M), np.float32)
    for c in range(NCORES):
        of = core_outs[c]  # (RPC, M, T2) bf16
        for r in range(RPC):
            b = c * RPC + r
            ep = epos[b]
            cols = of[r][:, np.where(ep >= 0, ep, 0)].T.astype(np.float32)
            final[b] = np.where((ep >= 0)[:, None], cols, b_eff[None, :])
    return final


def kernel(x, mask, tw_uniq, bn_gamma, bn_beta, bn_mean, bn_var,
           conv_w, conv_b, fc_w, fc_b):
    fsort, b01, epos, weffb, b_eff = _host_prep(
        x, mask, tw_uniq, bn_gamma, bn_beta, bn_mean, bn_var,
        conv_w, conv_b, fc_w, fc_b)

    if "nc" not in _CACHE:
        _CACHE["nc"] = build_bass()
    nc = _CACHE["nc"]

    in_maps = _build_in_maps(fsort, b01, weffb)
    res = bass_utils.run_bass_kernel_spmd(nc, in_maps, list(range(NCORES)))
    core_outs = [res.results[c]["out"] for c in range(NCORES)]
    return _unshard(core_outs, epos, b_eff)


# revision 17
# speedup vs baseline: 2.0703x; 1.0128x over previous
"""Trainium2 kernel for nn_AggrEncoder (segment-max + BN + 1x1 conv + fc).

Sharding: pure data-parallel over batch, 4 rows/core on 8 cores.

Host prep (layout only): per batch row, counting-sorts the valid (masked-in)
time columns by window id, pads each window's group to even length and splits
it across two half-arrays A|B at identical pair-positions, so a tensor_tensor
max (the "fold", 2x DVE throughput in bf16) halves the row to T2 pair-columns
with every window still contiguous. A and B are interleaved in DRAM at chunk
granularity (512 pair-cols for rows 0/3, 1024 for rows 1/2) so each fold
chunk depends on a single DMA transfer. BN+conv+fc fold into one (128->8)
affine W_eff/b_eff. All feature payload ships in bf16 (rel-err budget 2e-2;
measured ~6e-3).

Device pipeline (DVE is the bottleneck engine; the tile_wait_until stamps pin
the scheduler to the software-pipelined order below — its readiness model
otherwise hoists all folds ahead of the scans and starves DVE):
  1. fsort DMAs on the SP sequencer (row 0 in four transfers for a fast
     start), b01/weffb on the ACT sequencer.
  2. Fold on DVE per chunk: G = max(A_c, B_c).
  3. PE ones-matmul replicates pair-level segment-continuation flags into
     PSUM f32 per scan chunk; flag chunks for row r+1 run ahead of row r
     tails.
  4. DVE tensor_tensor_scan (op0=mult, op1=max) over G, flags read straight
     from PSUM, carry chained across chunks via `initial`. Flag=0 at window
     starts resets the carry and applies the reference max(0, .) clamp.
  5. Tail: W_eff^T @ S per 512-col PSUM bank; ACT scalar.add evacuates each
     bank (bias + f32->bf16) as soon as it is ready.
  6. out DMAs on SP; the last row's is split in half to shorten the drain.
Host unshard: picks each window's segment-end pair column; empty windows get
b_eff.
"""

import sys

import numpy as np

for _p in ("/opt/trn_rl_repo",):
    if _p not in sys.path:
        sys.path.insert(0, _p)

import concourse.bass as bass
import concourse.bacc as bacc
import concourse.mybir as mybir
from concourse import bass_utils
from concourse._compat import get_trn_type
from concourse.tile import TileContext

import ml_dtypes

B, T, D, Tu, Dout, M = 32, 4096, 128, 512, 64, 8
NCORES = 8
RPC = B // NCORES  # rows per core
BN_EPS = 1e-5

T2 = 2016          # pair columns per row (folded length)
BK = 512           # PSUM bank width in f32 — matmul output limit

# per-row interleave grain (pair columns per fold/scan chunk)
ROW_GRAIN = (504, 1008, 1008, 504)

_CACHE = {}


def build_bass():
    nc = bacc.Bacc(get_trn_type() or "TRN2", target_bir_lowering=False)

    fsort = nc.dram_tensor(
        "fsort", [RPC, D, 2 * T2], mybir.dt.bfloat16, kind="ExternalInput"
    )
    b01 = nc.dram_tensor(
        "b01", [1, RPC * T2], mybir.dt.bfloat16, kind="ExternalInput"
    )
    # cols 0:8 = W_eff^T (bf16); cols 8:10 = per-partition f32 bias (bitcast)
    weffb = nc.dram_tensor("weffb", [D, 10], mybir.dt.bfloat16, kind="ExternalInput")
    out = nc.dram_tensor(
        "out", [RPC, M, T2], mybir.dt.bfloat16, kind="ExternalOutput"
    )

    with TileContext(nc) as tc:
        with (
            tc.tile_pool(name="const", bufs=1) as cpool,
            tc.tile_pool(name="gpool", bufs=4) as gpool,
            tc.tile_pool(name="fpool", bufs=2) as fpool,
            tc.tile_pool(name="spool", bufs=2) as spool,
            tc.tile_pool(name="opool", bufs=2) as opool,
            tc.tile_pool(name="pb512", bufs=2, space="PSUM") as ppb5,
            tc.tile_pool(name="pb1024", bufs=2, space="PSUM") as ppb10,
            tc.tile_pool(name="psum_o", bufs=2, space="PSUM") as ppo,
        ):
            b01_sb = cpool.tile([1, RPC * T2], mybir.dt.bfloat16, tag="b01")
            nc.scalar.dma_start(b01_sb[:], b01[:])
            weffb_sb = cpool.tile([D, 10], mybir.dt.bfloat16, tag="weffb")
            ones_sb = cpool.tile([1, 128], mybir.dt.bfloat16, tag="ones")
            nc.vector.memset(ones_sb[:], 1.0)

            weff_ap = weffb_sb[:, 0:8]
            bias_ap = weffb_sb[:, 8:10].bitcast(mybir.dt.float32)[0:M]

            # ---- fsort DMAs (SP): row 0 in 4 transfers, rest in 2 ----
            FTs = []
            for r in range(RPC):
                FT = gpool.tile([D, 2 * T2], mybir.dt.bfloat16, tag="FT",
                                name=f"FT{r}")
                FTs.append(FT)
            Q0 = 2 * T2 // 4
            for c in range(4):
                lo = c * Q0
                nc.sync.dma_start(FTs[0][:, lo:lo + Q0],
                                  fsort[0][:, lo:lo + Q0])
            nc.sync.dma_start(weffb_sb[:], weffb[:])
            for r in range(1, RPC):
                nc.sync.dma_start(FTs[r][:, 0:T2], fsort[r][:, 0:T2])
                nc.sync.dma_start(FTs[r][:, T2:2 * T2], fsort[r][:, T2:2 * T2])

            # ---- per-row structures ----
            Gs, Ss, outs = [], [], []
            for r in range(RPC):
                G = fpool.tile([D, T2], mybir.dt.bfloat16, tag="G", name=f"G{r}")
                S = spool.tile([D, T2], mybir.dt.bfloat16, tag="S", name=f"S{r}")
                o = opool.tile([M, T2], mybir.dt.bfloat16, tag="osb",
                               name=f"osb{r}")
                Gs.append(G)
                Ss.append(S)
                outs.append(o)
            pbs = {}
            pos = {}

            def fold(r, c):
                gr = ROW_GRAIN[r]
                lo = c * gr
                nc.vector.tensor_tensor(
                    Gs[r][:, lo:lo + gr],
                    FTs[r][:, 2 * lo:2 * lo + gr],
                    FTs[r][:, 2 * lo + gr:2 * lo + 2 * gr],
                    op=mybir.AluOpType.max)

            def reps(r, c):
                gr = ROW_GRAIN[r]
                pool = ppb5 if gr == 512 else ppb10
                pb = pool.tile([128, gr], mybir.dt.float32, tag=f"pb{gr}",
                               name=f"pb{r}_{c}")
                pbs[(r, c)] = pb[:]
                for h0 in range(0, gr, BK):
                    w = min(BK, gr - h0)
                    fl_lo = r * T2 + c * gr + h0
                    nc.tensor.matmul(
                        pb[:, h0:h0 + w],
                        ones_sb[:],
                        b01_sb[:, fl_lo:fl_lo + w],
                        start=True, stop=True)

            def scan(r, c):
                gr = ROW_GRAIN[r]
                lo = c * gr
                nc.vector.tensor_tensor_scan(
                    Ss[r][:, lo:lo + gr],
                    pbs[(r, c)],
                    Gs[r][:, lo:lo + gr],
                    0.0 if c == 0 else Ss[r][:, lo - 1:lo],
                    op0=mybir.AluOpType.mult,
                    op1=mybir.AluOpType.max)

            TQ = T2 // 4
            def tail(r, k):
                po = ppo.tile([M, TQ], mybir.dt.float32, tag="po",
                              name=f"po{r}_{k}")
                pos[(r, k)] = po
                lo = k * TQ
                nc.tensor.matmul(
                    po[:], weff_ap, Ss[r][:, lo:lo + TQ],
                    start=True, stop=True)

            def evac(r, k):
                lo = k * TQ
                nc.scalar.add(outs[r][:, lo:lo + TQ], pos[(r, k)][:], bias_ap)

            def out_dma(r, h=None, eng=None):
                eng = eng or nc.sync
                if h is None:
                    eng.dma_start(out[r], outs[r][:])
                else:
                    lo = h * (T2 // 2)
                    eng.dma_start(out[r][:, lo:lo + T2 // 2],
                                  outs[r][:, lo:lo + T2 // 2])

            # software-pipelined emission, pinned via tile_wait_until.
            # DVE order keeps every op's producer >= 2 slots back so the
            # carry/fold dependency latency (~250ns) hides behind the
            # intervening op.
            steps = [
                lambda: fold(0, 0), lambda: reps(0, 0),
                lambda: fold(0, 1), lambda: reps(0, 1),
                lambda: scan(0, 0),
                lambda: fold(0, 2), lambda: reps(0, 2),
                lambda: scan(0, 1), lambda: tail(0, 0), lambda: evac(0, 0),
                lambda: fold(0, 3), lambda: reps(0, 3),
                lambda: scan(0, 2), lambda: tail(0, 1), lambda: evac(0, 1),
                lambda: fold(1, 0), lambda: reps(1, 0),
                lambda: scan(0, 3), lambda: tail(0, 2), lambda: evac(0, 2),
                lambda: fold(1, 1), lambda: reps(1, 1),
                lambda: scan(1, 0),
                lambda: tail(0, 3), lambda: evac(0, 3), lambda: out_dma(0),
                lambda: fold(2, 0), lambda: reps(2, 0),
                lambda: scan(1, 1), lambda: tail(1, 0), lambda: evac(1, 0),
                lambda: fold(2, 1), lambda: reps(2, 1),
                lambda: scan(2, 0), lambda: tail(1, 1), lambda: evac(1, 1),
                lambda: tail(1, 2), lambda: evac(1, 2),
                lambda: fold(3, 0), lambda: reps(3, 0),
                lambda: scan(2, 1),
                lambda: tail(1, 3), lambda: evac(1, 3), lambda: out_dma(1),
                lambda: tail(2, 0), lambda: evac(2, 0),
                lambda: tail(2, 1), lambda: evac(2, 1),
                lambda: fold(3, 1), lambda: reps(3, 1),
                lambda: scan(3, 0), lambda: tail(2, 2), lambda: evac(2, 2),
                lambda: fold(3, 2), lambda: reps(3, 2),
                lambda: scan(3, 1),
                lambda: tail(2, 3), lambda: evac(2, 3), lambda: out_dma(2),
                lambda: fold(3, 3), lambda: reps(3, 3),
                lambda: scan(3, 2), lambda: tail(3, 0), lambda: evac(3, 0),
                lambda: tail(3, 1), lambda: evac(3, 1), lambda: out_dma(3, 0),
                lambda: scan(3, 3), lambda: tail(3, 2), lambda: evac(3, 2),
                lambda: tail(3, 3), lambda: evac(3, 3), lambda: out_dma(3, 1),
            ]
            import os
            force = os.environ.get("KW_FORCE", "1") == "1"
            for i, fn in enumerate(steps):
                with tc.tile_wait_until(i * 0.02, enable=force):
                    fn()

    if not nc.is_finalized():
        nc.finalize()
    return nc


def _host_prep(x, mask, tw_uniq, bn_gamma, bn_beta, bn_mean, bn_var,
               conv_w, conv_b, fc_w, fc_b):
    tw = x[:, :, 0]
    u0 = tw_uniq[:, 0, 0]
    idx = np.clip((tw - u0[:, None]).astype(np.int32), 0, Tu - 1)  # (B, T)
    valid = mask[:, :, 0]
    key = np.where(valid, idx, Tu).astype(np.int64)                # (B, T)

    featsT = np.ascontiguousarray(
        x[:, :, 1:].transpose(0, 2, 1)).astype(ml_dtypes.bfloat16)  # (B, D, T)

    fsort = np.zeros((B, D, 2 * T2), ml_dtypes.bfloat16)
    b01 = np.ones((B, T2), np.float32)
    epos = np.empty((B, Tu), np.int64)

    rowidx = np.arange(T)
    for b in range(B):
        gr = ROW_GRAIN[b % RPC]
        counts = np.bincount(key[b], minlength=Tu + 1)[:Tu]        # (Tu,)
        h = (counts + 1) // 2
        hrow = int(h.sum())
        if hrow > T2:
            # overflow fallback (not expected for the reference's mask):
            # reduce this row host-side and ship one column per window.
            hidden = np.zeros((D, Tu), np.float32)
            fv = featsT[b].astype(np.float32)
            v = valid[b]
            np.maximum.at(hidden.T, idx[b][v], fv[:, v].T)
            hb = np.maximum(hidden, 0.0).astype(ml_dtypes.bfloat16)
            fsort[b] = 0
            # A-columns of each chunk hold the values; B-columns stay 0
            pc = np.arange(Tu)
            acol = 2 * gr * (pc // gr) + (pc % gr)
            fsort[b][:, acol] = hb
            b01[b] = 1.0
            b01[b][:Tu] = 0.0
            epos[b] = np.where(counts > 0, np.arange(Tu), -1)
            continue

        order = np.argsort(key[b], kind="stable")                  # (T,)
        skey = key[b][order]
        nvalid = int((skey < Tu).sum())
        order = order[:nvalid]
        skey = skey[:nvalid]

        cstart = np.concatenate([[0], np.cumsum(counts)])          # (Tu+1,)
        rank = rowidx[:nvalid] - cstart[skey]                      # pos in segment
        pstart = np.concatenate([[0], np.cumsum(h)[:-1]])          # (Tu,)
        hseg = h[skey]
        in_a = rank < hseg
        paircol = pstart[skey] + np.where(in_a, rank, rank - hseg)
        base = 2 * gr * (paircol // gr) + (paircol % gr)
        col = np.where(in_a, base, base + gr)

        fsort[b][:, col] = featsT[b][:, order]
        b01[b][pstart] = 0.0
        b01[b][hrow:] = 0.0
        epos[b] = np.where(counts > 0, pstart + h - 1, -1)

    s = (bn_gamma.astype(np.float64)
         / np.sqrt(bn_var.astype(np.float64) + BN_EPS))
    t_aff = bn_beta.astype(np.float64) - bn_mean.astype(np.float64) * s
    wc = fc_w.astype(np.float64) @ conv_w.astype(np.float64)       # (8, 128)
    w_eff = wc * s[None, :]
    b_eff = (fc_w.astype(np.float64)
             @ (conv_w.astype(np.float64) @ t_aff + conv_b.astype(np.float64))
             + fc_b.astype(np.float64)).astype(np.float32)         # (8,)

    # weffb: [128, 10] bf16; cols 0:8 = W_eff^T, cols 8:10 = f32 bias bitcast
    weffb = np.zeros((D, 10), ml_dtypes.bfloat16)
    weffb[:, 0:8] = w_eff.T.astype(ml_dtypes.bfloat16)
    bias128 = np.zeros((D,), np.float32)
    bias128[:M] = b_eff
    weffb[:, 8:10] = bias128.reshape(D, 1).view(ml_dtypes.bfloat16)

    return fsort, b01.astype(ml_dtypes.bfloat16), epos, weffb, b_eff


def _build_in_maps(fsort, b01, weffb):
    in_maps = []
    for c in range(NCORES):
        rows = slice(c * RPC, (c + 1) * RPC)
        in_maps.append(dict(
            fsort=fsort[rows],
            b01=b01[rows].reshape(1, RPC * T2),
            weffb=weffb,
        ))
    return in_maps


def _unshard(core_outs, epos, b_eff):
    final = np.empty((B, Tu, M), np.float32)
    for c in range(NCORES):
        of = core_outs[c]  # (RPC, M, T2) bf16
        for r in range(RPC):
            b = c * RPC + r
            ep = epos[b]
            cols = of[r][:, np.where(ep >= 0, ep, 0)].T.astype(np.float32)
            final[b] = np.where((ep >= 0)[:, None], cols, b_eff[None, :])
    return final


def kernel(x, mask, tw_uniq, bn_gamma, bn_beta, bn_mean, bn_var,
           conv_w, conv_b, fc_w, fc_b):
    fsort, b01, epos, weffb, b_eff = _host_prep(
        x, mask, tw_uniq, bn_gamma, bn_beta, bn_mean, bn_var,
        conv_w, conv_b, fc_w, fc_b)

    if "nc" not in _CACHE:
        _CACHE["nc"] = build_bass()
    nc = _CACHE["nc"]

    in_maps = _build_in_maps(fsort, b01, weffb)
    res = bass_utils.run_bass_kernel_spmd(nc, in_maps, list(range(NCORES)))
    core_outs = [res.results[c]["out"] for c in range(NCORES)]
    return _unshard(core_outs, epos, b_eff)
